# revision 26
# baseline (speedup 1.0000x reference)
"""Trainium2 Bass kernel for nn_FIB_RNN (GRU encoder + autoregressive
sampling decoder with DenseVariational head).

Contract: kernel(**inputs) takes the FULL unsharded inputs (numpy arrays,
keys as in reference.setup_inputs()) and returns the FULL output
[B, GAMMA, 2] float32.

Strategy: pure data parallelism over the batch dim across 8 NeuronCores
(1024 batch rows per core).  Within a core the GRU state is kept
feature-major [U=128 partitions, batch free] so the recurrent matmul is
lhsT=R_gate[128,128] @ rhs=h[128,512] -> PSUM, and the scalar sequence
input enters as a K=1 outer-product matmul accumulated into the same
PSUM bank.  The tiny DenseVariational weights are sampled on the host
(deterministic given dv_eps) and folded into per-step [128,1] matmuls.

v7 structure (875us baseline -> 743us -> 662us):
 - scheduler realignment: the Tile scheduler's cost model assumed the PE
   warms to 2.4 GHz, but the HAM never grants it on this kernel; pinning
   the modeled clock to 1.2 GHz (KERNEL_SCHED_COLD_PE=1) fixed the ACT
   queue order (tanh was head-of-line blocked behind the other chunk's
   sigmoids) and cut the encoder from 8.1 to 6.9 us/step.
 - dense head on 2 PSUM partitions: zero-padded lhsT pairs accumulate
   chunk c's loc/scale pre-acts into row c*32 of one [33, CW] PSUM tile
   (matmul base partitions must be 0/32/64, and lhsT/rhs must share the
   base — K row replicated to partition 32 for the chunk-1 K@y matmul).
   sigmoid/ln/loc/AMR/y-add each handle both chunks in ONE FD=512
   instruction, cutting ~2us off the serial sampling tail per step.

v3 structure (vs the v1 baseline, 875us -> 743us):
 - bf16 matmul operands + h-state (default KERNEL_MM_DT=bf16): the PE
   streams 2 cols/cycle and the d/e/h2 update chain hits the DVE 2x_1P
   packed mode (379ns vs 614-666ns per op).  Measured rel err 1.30e-2
   vs the 2e-2 gate; KERNEL_MM_DT=f32r is the high-precision fallback.
 - scale output finished on the HOST: the kernel ships ln(g) rows; the
   affine 1e-5 - 0.05*ln(g) is applied in numpy after the gather.
   Removes 56 Identity acts + 56 row DMAs from the decoder loop.
 - decoder emission reordered: next step's R@h matmuls are enqueued
   before the dense head's row ops, so the PE works through the serial
   sampling tail; K@y matmuls issue per chunk right after that chunk's
   y row is ready.
 - loc rows on the DVE (tensor_scalar from PSUM) so the ACT queue is
   free for the sigmoid->ACT_TABLE_LOAD->Ln->reload chain.

Negative results (tried, reverted): PE-warming dummy matmuls (fill HAM
activity window with junk MMs into a scratch PSUM bank): warming IS
real (matmul avg 475->417ns, warm bursts at 215-258ns) but 10 dummies
per step block the in-order PE queue longer than the warming saves
(743us vs 662) - a tuned count ~4-5/step sized to the warm-case idle
gap is the most promising unexplored lead; GpSimd CANNOT access PSUM (BIR
verifier: "GPSIMD Instructions cannot access PSUM") - no pool STT on
matmul outputs; tt/uu in bf16 (encoder STT 2x candidate) pushes sim rel
err 9.1e-3 -> 1.31e-2, projected HW ~1.9e-2 vs the 2e-2 gate - too
thin; e/h2 on the GpSimd engine (Pool TT
ops cost ~1.4us and sit on the recurrent critical chain); tanh as
2*sigmoid(2x)-1 via LN_BWD_DX_ANT (795ns vs 614ns plain sub); a custom
PWP act-table root merging ln into sigmoid_and_others (ctrl-word format
is (cfg<<11)|bucket_base and the merge fits 1536 buckets, but NRT
rejects the NEFF at load - the runtime has its own baked table images).
KERNEL_TABLE_MERGE=1 keeps that experiment reachable.
"""

import json
import os
import shutil
import sys
from contextlib import ExitStack
from pathlib import Path

import numpy as np

for _p in ("/opt/trn_rl_repo", "/root/.axon_site/_ro/trn_rl_repo"):
    if os.path.isdir(_p) and _p not in sys.path:
        sys.path.insert(0, _p)

F32 = None  # set after imports
U = 128                    # rnn units
T_ENC = 48                 # encoder steps
GAMMA = 28                 # decoder outputs (27 sampled feedback steps)
N_CORES = 8
B_FULL = 8192
BC = B_FULL // N_CORES     # 1024 batch rows per core
CW = 512                   # chunk width (PSUM bank = 512 fp32)
NCH = BC // CW             # 2 chunks per core
RP = 32                    # dense-head row stride (matmul base-partition rule)
RN = RP + 1                # dense-head tile partition count (rows 0 and RP live)
C_SP = float(np.log(np.expm1(1.0)))  # softplus^-1(1.0)
Q_SCALE = 0.02
OP_SCALE = 0.05

_MERGE_TABLES = os.environ.get("KERNEL_TABLE_MERGE", "0") == "1"

# ---------------------------------------------------------------------------
# Custom activation-table root: sigmoid_and_others + ln merged.
# ---------------------------------------------------------------------------

_PWP_SRC = (
    "/nix/store/z022hj2nvbm3nwdizlisq4ylc0y7rd6q-python3-3.13.14-env/"
    "lib/python3.13/site-packages/neuronxcc/pwp/pwp_bin_trainium"
)
_DROP_FUNCS = ("tanh", "erf", "arctan")
_LN_EXP_LO, _LN_EXP_HI = -32, 1   # keep ln buckets for input exp in [lo, hi]


def _build_merged_act_root(dst: str) -> str:
    """Create a PWP act root at dst where sigmoid_and_others also serves
    ln.  Ctrl entries are (cfg<<11)|bucket_base words; rebasing = adjust
    the low 11 bits.  Returns path to the new act_info.json."""
    src = Path(_PWP_SRC)
    dstp = Path(dst)
    if (dstp / "act_info.json").exists():
        return str(dstp / "act_info.json")
    dstp.mkdir(parents=True, exist_ok=True)
    info = json.loads((src / "act_info.json").read_text())

    for ent in info["act_func_sets"]:
        if ent["name"] != "sigmoid_and_others":
            for k in ("bkt_bin", "ctrl_bin", "profile_json"):
                if not (dstp / ent[k]).exists():
                    shutil.copy(src / ent[k], dstp / ent[k])
            continue

        prof = json.loads((src / ent["profile_json"]).read_text())
        bkt = np.fromfile(src / ent["bkt_bin"], dtype=np.uint32).reshape(-1, 8)
        ctl = np.fromfile(src / ent["ctrl_bin"], dtype=np.uint32).reshape(-1, 8)
        nlj = json.loads((src / "natural_log.json").read_text())
        nl_bkt = np.fromfile(src / "natural_log_bkt.bin", dtype=np.uint32).reshape(-1, 8)
        nl_ctl = np.fromfile(src / "natural_log_ctrl.bin", dtype=np.uint32).reshape(-1, 8)

        funcs = list(prof["func_to_bkt_start_idx"].keys())
        n_bkt, n_ctl = prof["bkt_entry_cnt"], prof["ctl_entry_cnt"]
        bkt_start = dict(prof["func_to_bkt_start_idx"])
        ctl_start = dict(prof["func_to_ctl_start_idx"])
        order = sorted(funcs, key=lambda f: bkt_start[f])

        def _blk(d, f, total):
            o = sorted(funcs, key=lambda x: d[x])
            i = o.index(f)
            end = d[o[i + 1]] if i + 1 < len(o) else total
            return d[f], end

        keep = [f for f in order if f not in _DROP_FUNCS]
        new_bkt_rows, new_ctl_rows = [], []
        new_bs, new_cs = {}, {}
        meta_by_name = {m["func_name"]: m for m in prof["profile_meta_data"]}
        new_meta = []
        new_exp_bkt, new_exp_ctl = {}, {}
        for f in keep:
            b0, b1 = _blk(bkt_start, f, n_bkt)
            c0, c1 = _blk(ctl_start, f, n_ctl)
            nb, ncs = len(new_bkt_rows), len(new_ctl_rows)
            new_bs[f] = nb
            new_cs[f] = ncs
            new_bkt_rows.extend(bkt[b0:b1])
            blkc = ctl[c0:c1].copy()
            # rebase bucket pointers: low 11 bits
            base = blkc[:, 0] & 0x7FF
            cfg = blkc[:, 0] & ~np.uint32(0x7FF)
            blkc[:, 0] = cfg | ((base - b0 + nb) & 0x7FF)
            new_ctl_rows.extend(blkc)
            eb = prof.get("func_exp_to_bkt_start_idx", {}).get(f, {})
            new_exp_bkt[f] = {
                e: [v - b0 + nb for v in vs] for e, vs in eb.items()
            }
            ec = prof.get("func_exp_to_ctl_start_idx", {}).get(f, {})
            new_exp_ctl[f] = {
                e: [v - c0 + ncs for v in vs] for e, vs in ec.items()
            }
            mm = [m for m in prof["profile_meta_data"]
                  if m["func_name"].rsplit("_", 1)[0] == f]
            new_meta.extend(mm)

        # ---- append trimmed ln from natural_log ----
        ln_exp_bkt = nlj["func_exp_to_bkt_start_idx"]["ln"]
        ln_exp_ctl = nlj["func_exp_to_ctl_start_idx"]["ln"]
        exps = sorted(int(e) for e in ln_exp_bkt)
        keep_exps = [e for e in exps if _LN_EXP_LO <= e <= _LN_EXP_HI]
        lo_bkt = min(ln_exp_bkt[str(e)][0] for e in keep_exps)
        hi_bkt = max(
            (ln_exp_bkt[str(e + 1)][0] if str(e + 1) in ln_exp_bkt else None)
            or nlj["func_to_bkt_start_idx"]["relu"]
            for e in keep_exps
        )
        lo_ctl = min(ln_exp_ctl[str(e)][0] for e in keep_exps)
        hi_ctl = max(
            (ln_exp_ctl[str(e + 1)][0] if str(e + 1) in ln_exp_ctl else None)
            or nlj["func_to_ctl_start_idx"]["relu"]
            for e in keep_exps
        )
        nb, ncs = len(new_bkt_rows), len(new_ctl_rows)
        assert nb + (hi_bkt - lo_bkt) <= 1536, (nb, hi_bkt - lo_bkt)
        new_bs["ln"] = nb
        new_cs["ln"] = ncs
        new_bkt_rows.extend(nl_bkt[lo_bkt:hi_bkt])
        blkc = nl_ctl[lo_ctl:hi_ctl].copy()
        base = blkc[:, 0] & 0x7FF
        cfg = blkc[:, 0] & ~np.uint32(0x7FF)
        blkc[:, 0] = cfg | ((base - lo_bkt + nb) & 0x7FF)
        new_ctl_rows.extend(blkc)
        new_exp_bkt["ln"] = {
            str(e): [v - lo_bkt + nb for v in ln_exp_bkt[str(e)]]
            for e in keep_exps
        }
        new_exp_ctl["ln"] = {
            str(e): [v - lo_ctl + ncs for v in ln_exp_ctl[str(e)]]
            for e in keep_exps
        }
        ln_meta = [m for m in nlj["profile_meta_data"]
                   if m["func_name"].startswith("ln")]
        assert len(ln_meta) == 1
        lm = dict(ln_meta[0])
        # remap ln's control bases (ctl indices) by the ctl shift
        for key in ("pwl_control_base_pos", "pwl_control_base_neg",
                    "pos_small_signal_pwl_control", "neg_small_signal_pwl_control",
                    "pos_large_signal_pwl_control", "neg_large_signal_pwl_control"):
            if key in lm and isinstance(lm[key], int):
                lm[key] = lm[key] - lo_ctl + ncs
        # clamp bounds to the kept bucket range: [2^lo_exp, 2^(hi_exp+1))
        lm["lower_bound"] = int(
            np.float32(2.0 ** _LN_EXP_LO).view(np.uint32)
        )
        lm["upper_bound"] = int(
            np.nextafter(np.float32(2.0 ** (_LN_EXP_HI + 1)),
                         np.float32(0.0)).view(np.uint32)
        )
        new_meta.append(lm)

        bkt_arr = np.vstack(new_bkt_rows).astype(np.uint32)
        ctl_arr = np.vstack(new_ctl_rows).astype(np.uint32)
        bkt_arr.tofile(dstp / ent["bkt_bin"])
        ctl_arr.tofile(dstp / ent["ctrl_bin"])
        prof2 = dict(prof)
        prof2["bkt_entry_cnt"] = int(bkt_arr.shape[0])
        prof2["ctl_entry_cnt"] = int(ctl_arr.shape[0])
        prof2["func_to_bkt_start_idx"] = new_bs
        prof2["func_to_ctl_start_idx"] = new_cs
        prof2["func_exp_to_bkt_start_idx"] = new_exp_bkt
        prof2["func_exp_to_ctl_start_idx"] = new_exp_ctl
        prof2["profile_meta_data"] = new_meta
        (dstp / ent["profile_json"]).write_text(json.dumps(prof2))
        act = {f: n for f, n in ent["act"].items() if f not in _DROP_FUNCS}
        act["ln"] = 40
        ent["act"] = act

    (dstp / "act_info.json").write_text(json.dumps(info))
    return str(dstp / "act_info.json")


if _MERGE_TABLES and os.path.isdir(_PWP_SRC):
    _root = _build_merged_act_root("/tmp/kernel_act_root_v1")
    os.environ["BASS_ACT_ROOT_JSON_PATH"] = _root
else:
    _MERGE_TABLES = False

import concourse.bass as bass
import concourse.tile as tile
from concourse import bacc, mybir
from concourse.bass_utils import run_bass_kernel_spmd
from concourse.dve_ops import AFFINE_MUL_REDUCE, LN_BWD_DX_ANT

F32 = mybir.dt.float32
AF = mybir.ActivationFunctionType
ALU = mybir.AluOpType

# The Tile scheduler's cost model assumes the PE ramps to 2.4 GHz after
# 3us of continuous work; on this kernel the HAM never grants the warm
# clock (measured: all matmuls at ~1.2 GHz).  Pinning the modeled warm
# clock to 1.2 GHz realigns the scheduler's readiness estimates with
# reality (pure scheduling hint; no semantic change).
if os.environ.get("KERNEL_SCHED_COLD_PE", "1") == "1":
    from concourse.hw_specs import TRN2Spec as _Spec

    _Spec.PE_CYCLE = 1e9 / 1.2e9

if _MERGE_TABLES:
    # keep the bass-side view of the table sets consistent with the
    # custom act root (the returned dict is cached; mutate in place).
    from concourse.hw_specs import get_activation_tables

    _tabs = get_activation_tables("gen3")
    _sig = _tabs["sigmoid_and_others"]
    _sig.add(AF.Ln)
    _sig.discard(AF.Tanh)
    for _af in ("Erf", "Arctan"):
        if hasattr(AF, _af):
            _sig.discard(getattr(AF, _af))

# matmul operand dtype.  float32r (e8m11-rounded fp32) streams 1 col/cycle
# vs 4 cycles/col for full fp32; bfloat16 streams 2 cols/cycle.
_MM_MODE = os.environ.get("KERNEL_MM_DT", "bf16")
RD = {"f32r": mybir.dt.float32r, "bf16": mybir.dt.bfloat16, "f32": F32}[_MM_MODE]
RD16 = mybir.dt.bfloat16 if _MM_MODE == "bf16" else F32

_CACHE = {}


def _round_fp32r(a):
    a = np.ascontiguousarray(a, np.float32)
    if _MM_MODE == "f32":
        return a
    if _MM_MODE == "bf16":
        import ml_dtypes
        return np.ascontiguousarray(a.astype(ml_dtypes.bfloat16))
    bits = a.view(np.uint32)
    out = ((bits.astype(np.uint64) + 0x800) & 0xFFFFF000).astype(np.uint32)
    return out.view(np.float32)


def _build_program(with_b1h):
    nc = bacc.Bacc("TRN2", target_bir_lowering=False, debug=False)

    x_seq = nc.dram_tensor("x_seq", [T_ENC, BC], RD, kind="ExternalInput").ap()
    eps_seq = nc.dram_tensor("eps_seq", [GAMMA - 1, BC], F32, kind="ExternalInput").ap()
    r_w = nc.dram_tensor("r_w", [U, 3 * U], RD, kind="ExternalInput").ap()
    k_w = nc.dram_tensor("k_w", [RN, 3 * U], RD, kind="ExternalInput").ap()
    k_col = nc.dram_tensor("k_col", [U, 3], F32, kind="ExternalInput").ap()
    wkd0 = nc.dram_tensor("wkd0", [U, 2 * RN * GAMMA], RD, kind="ExternalInput").ap()
    wkd1 = nc.dram_tensor("wkd1", [U, 2 * RN * GAMMA], RD, kind="ExternalInput").ap()
    wb0 = nc.dram_tensor("wb0", [RN, GAMMA], F32, kind="ExternalInput").ap()
    cb1 = nc.dram_tensor("cb1", [RN, GAMMA], F32, kind="ExternalInput").ap()
    gb = nc.dram_tensor("gb", [U, 4], F32, kind="ExternalInput").ap()
    h0_z = nc.dram_tensor("h0_z", [U, BC], RD, kind="ExternalInput").ap()
    loc_fm = nc.dram_tensor("loc_fm", [GAMMA, BC], F32, kind="ExternalOutput").ap()
    sp_fm = nc.dram_tensor("sp_fm", [GAMMA, BC], F32, kind="ExternalOutput").ap()

    with tile.TileContext(nc) as tc, ExitStack() as es:
        consts = es.enter_context(tc.tile_pool(name="consts", bufs=1))
        R = consts.tile([U, 3 * U], RD)
        K = consts.tile([RN, 3 * U], RD)
        KC = consts.tile([U, 3], F32)
        WKD0 = consts.tile([U, 2 * RN * GAMMA], RD)
        WKD1 = consts.tile([U, 2 * RN * GAMMA], RD)
        WB0 = consts.tile([RN, GAMMA], F32)
        CB1 = consts.tile([RN, GAMMA], F32)
        GB = consts.tile([U, 4], F32)
        EPT = consts.tile([RN, CW], F32)
        nc.vector.memset(EPT[:], 0.0)
        nc.sync.dma_start(R[:], r_w[:])
        nc.sync.dma_start(K[:], k_w[:])
        nc.sync.dma_start(KC[:], k_col[:])
        nc.sync.dma_start(WKD0[:], wkd0[:])
        nc.sync.dma_start(WKD1[:], wkd1[:])
        nc.sync.dma_start(WB0[:], wb0[:])
        nc.sync.dma_start(CB1[:], cb1[:])
        nc.sync.dma_start(GB[:], gb[:])

        hpool = es.enter_context(tc.tile_pool(name="h", bufs=4))
        gates = es.enter_context(tc.tile_pool(name="gates", bufs=3))
        samp = es.enter_context(tc.tile_pool(name="samp", bufs=2))
        stage = es.enter_context(tc.tile_pool(name="stage", bufs=5))
        ps_g = es.enter_context(tc.tile_pool(
            name="psg", bufs=int(os.environ.get("KERNEL_PS_BUFS", "8")),
            space="PSUM"))

        hT = hpool.tile([U, BC], RD, tag="h")
        nc.sync.dma_start(hT[:], h0_z[:])
        hbox = [hT, None]  # [current h, next h being written]

        z3 = bass.ts(0, U)
        r3 = bass.ts(1, U)
        h3 = bass.ts(2, U)

        def gru_elem(c, psr, psh, psz, psx, xb):
            """Post-matmul element chain for chunk c ([U,CW] PSUM tiles).
            Either xb (encoder broadcast tile; x*K_h enters via STT) or
            psx (decoder; K_h@y PSUM)."""
            cs = bass.ts(c, CW)
            hc = hbox[0][:, cs]
            # r = sigmoid(rho + br)                     (GB1 = br)
            r_ = gates.tile([U, CW], RD16, tag=f"r_{c}")
            nc.scalar.activation(r_[:], psr[:], AF.Sigmoid,
                                 bias=GB[:, 1:2], scale=1.0)
            hrec = psh[:]
            if with_b1h:
                hb = gates.tile([U, CW], F32, tag=f"hb_{c}")
                nc.vector.tensor_scalar(
                    hb[:], psh[:], GB[:, 3:4], None, op0=ALU.add
                )
                hrec = hb[:]
            tt = gates.tile([U, CW], F32, tag=f"t_{c}")
            nc.vector.tensor_mul(tt[:], r_[:], hrec)
            uu = gates.tile([U, CW], F32, tag=f"u_{c}")
            if xb is not None:
                nc.vector.scalar_tensor_tensor(
                    uu[:], xb[:, cs], KC[:, 2:3], tt[:],
                    op0=ALU.mult, op1=ALU.add,
                )
            else:
                nc.vector.tensor_add(uu[:], tt[:], psx[:])
            hh = gates.tile([U, CW], RD16, tag=f"hh_{c}")
            nc.scalar.activation(hh[:], uu[:], AF.Tanh, bias=GB[:, 2:3],
                                 scale=1.0)
            # u1 = 1-z = sigmoid(-(zeta + bz))  (GB0 = -bz).  Emitted after
            # the tanh: u1 is needed only by e, and an earlier emission lets
            # the scheduler queue-block the tanh behind the other chunk's
            # sigmoids on the in-order ACT engine.
            u1 = gates.tile([U, CW], RD16, tag=f"u1_{c}")
            nc.scalar.activation(u1[:], psz[:], AF.Sigmoid,
                                 bias=GB[:, 0:1], scale=-1.0)
            # h' = h + (1-z)*(hh - h): three consecutive DVE ops (bf16
            # operands hit the 2x_1P packed mode)
            d = gates.tile([U, CW], RD16, tag=f"d_{c}")
            nc.vector.tensor_sub(d[:], hh[:], hc)
            e = gates.tile([U, CW], RD16, tag=f"e_{c}")
            nc.vector.tensor_mul(e[:], u1[:], d[:])
            nc.vector.tensor_add(hbox[1][:, cs], hc, e[:])

        # ---- encoder: 48 GRU steps ----
        for t in range(T_ENC):
            xb = stage.tile([U, BC], RD, tag="xb")
            nc.sync.dma_start(xb[:], x_seq[t : t + 1, :].partition_broadcast(U))
            hT = hbox[0]
            h_next = hpool.tile([U, BC], RD, tag="h")
            hbox[1] = h_next
            for c in range(NCH):
                cs = bass.ts(c, CW)
                x_row = xb[0:1, cs]
                psr = ps_g.tile([U, CW], F32, tag="ps")
                psh = ps_g.tile([U, CW], F32, tag="ps")
                psz = ps_g.tile([U, CW], F32, tag="ps")
                nc.tensor.matmul(psr[:], K[0:1, r3], x_row, start=True, stop=False)
                nc.tensor.matmul(psr[:], R[:, r3], hT[:, cs], start=False, stop=True)
                nc.tensor.matmul(psh[:], R[:, h3], hT[:, cs], start=True, stop=True)
                nc.tensor.matmul(psz[:], K[0:1, z3], x_row, start=True, stop=False)
                nc.tensor.matmul(psz[:], R[:, z3], hT[:, cs], start=False, stop=True)
                gru_elem(c, psr, psh, psz, None, xb)
            hbox[0] = hbox[1]

        # ---- decoder ----
        def dense_mms(t):
            """Dense head matmuls for step t.  Both chunks land in one
            [2, CW] PSUM tile (row c = chunk c) via zero-padded lhsT
            columns accumulated across two matmuls, so every downstream
            row op handles both chunks in a single instruction."""
            hT = hbox[0]
            psl = ps_g.tile([RN, CW], F32, tag="ps")
            pss = ps_g.tile([RN, CW], F32, tag="ps")
            for c, WKD in ((0, WKD0), (1, WKD1)):
                cs = bass.ts(c, CW)
                o = 2 * RN * t
                nc.tensor.matmul(psl[:], WKD[:, o : o + RN], hT[:, cs],
                                 start=(c == 0), stop=(c == 1))
                nc.tensor.matmul(pss[:], WKD[:, o + RN : o + 2 * RN],
                                 hT[:, cs], start=(c == 0), stop=(c == 1))
            return psl, pss

        def head_rows(t, ps):
            """sigmoid+ln+loc rows for step t ([2, CW]: row c = chunk c);
            DMA out; returns (loc, sp)."""
            psl, pss = ps
            g = samp.tile([RN, CW], F32, tag="g")
            loc = samp.tile([RN, CW], F32, tag="loc")
            # g = sigmoid(-(s + C + wb1))   (CB1 = -(C+wb1))
            nc.scalar.activation(
                g[:], pss[:], AF.Sigmoid, bias=CB1[:, t : t + 1],
                scale=-1.0,
            )
            # loc = h@W0 + wb0   (DVE: keeps the ACT queue free for the
            # table-load + Ln chain)
            nc.vector.tensor_scalar(
                loc[:], psl[:], WB0[:, t : t + 1], None, op0=ALU.add,
            )
            # sp = ln(g) = -softplus  (host applies 1e-5 - 0.05*sp)
            sp = samp.tile([RN, CW], F32, tag="sp")
            nc.scalar.activation(sp[:], g[:], AF.Ln, bias=0.0, scale=1.0)
            for c in range(NCH):
                cs = bass.ts(c, CW)
                rr = c * RP
                nc.sync.dma_start(loc_fm[t : t + 1, cs], loc[rr : rr + 1, :])
                nc.sync.dma_start(sp_fm[t : t + 1, cs], sp[rr : rr + 1, :])
            return loc, sp

        ps = dense_mms(0)
        for t in range(1, GAMMA):
            # next step's recurrent matmuls first: PE works through the tail
            hT = hbox[0]
            gps = []
            for c in range(NCH):
                cs = bass.ts(c, CW)
                psr = ps_g.tile([U, CW], F32, tag="ps")
                psh = ps_g.tile([U, CW], F32, tag="ps")
                psz = ps_g.tile([U, CW], F32, tag="ps")
                psx = ps_g.tile([U, CW], F32, tag="ps")
                nc.tensor.matmul(psr[:], R[:, r3], hT[:, cs], start=True,
                                 stop=False)
                nc.tensor.matmul(psh[:], R[:, h3], hT[:, cs], start=True,
                                 stop=True)
                nc.tensor.matmul(psz[:], R[:, z3], hT[:, cs], start=True,
                                 stop=False)
                gps.append((psr, psh, psz, psx))
            loc, sp = head_rows(t - 1, ps)
            # sample: y = loc + (1e-5 - 0.05*sp) * eps, rows {0, RP}
            ep = EPT
            for c in range(NCH):
                cs = bass.ts(c, CW)
                rr = c * RP
                nc.sync.dma_start(ep[rr : rr + 1, :], eps_seq[t - 1 : t, cs])
            m = samp.tile([RN, CW], F32, tag="m")
            nc.vector._custom_dve(
                AFFINE_MUL_REDUCE, out=m[:], in0=sp[:], in1=ep[:],
                s0=-OP_SCALE, s1=1e-5,
            )
            y = samp.tile([RN, CW], RD, tag="y")
            nc.vector.tensor_add(y[:], m[:], loc[:])
            h_next = hpool.tile([U, BC], RD, tag="h")
            hbox[1] = h_next
            for c in range(NCH):
                psr, psh, psz, psx = gps[c]
                rr = c * RP
                y_row = y[rr : rr + 1, :]
                Kc = K[rr : rr + 1, :]
                nc.tensor.matmul(psr[:], Kc[:, r3], y_row, start=False,
                                 stop=True)
                nc.tensor.matmul(psx[:], Kc[:, h3], y_row, start=True,
                                 stop=True)
                nc.tensor.matmul(psz[:], Kc[:, z3], y_row, start=False,
                                 stop=True)
                gru_elem(c, psr, psh, psz, psx, None)
            hbox[0] = hbox[1]
            ps = dense_mms(t)
        head_rows(GAMMA - 1, ps)

    nc.compile()
    return nc


def _k_rows(gru_kernel):
    k = np.zeros((RN, 3 * U), np.float32)
    k[0] = np.asarray(gru_kernel, np.float32)[0]
    k[RP] = k[0]
    return k


def _host_prep(inputs, gru_kernel, gru_rec_kernel, gru_bias, dv_loc, dv_rho,
               dv_eps, samp_eps):
    inputs = np.asarray(inputs, np.float32)
    B = inputs.shape[0]
    assert B == B_FULL, f"kernel compiled for B={B_FULL}, got {B}"
    xT = _round_fp32r(inputs[:, :T_ENC, 0].T)                  # [48, B]
    epsT = np.ascontiguousarray(np.asarray(samp_eps, np.float32)[:, :, 0])

    gru_bias = np.asarray(gru_bias, np.float32)
    b0, b1 = gru_bias[0], gru_bias[1]
    gb = np.zeros((U, 4), np.float32)
    gb[:, 0] = -(b0[0:U] + b1[0:U])
    gb[:, 1] = b0[U : 2 * U] + b1[U : 2 * U]
    gb[:, 2] = b0[2 * U : 3 * U]
    gb[:, 3] = b1[2 * U : 3 * U]

    dv_loc = np.asarray(dv_loc, np.float32)
    dv_rho = np.asarray(dv_rho, np.float32)
    dv_eps = np.asarray(dv_eps, np.float32)
    scale_q = np.float32(1e-5) + np.float32(Q_SCALE) * np.logaddexp(
        np.float32(C_SP) + dv_rho, np.float32(0.0), dtype=np.float32
    )
    w_all = dv_loc[None, :] + scale_q[None, :] * dv_eps        # [28, 258]
    wk = w_all[:, : 2 * U].reshape(GAMMA, U, 2).transpose(1, 0, 2)  # [U,28,2]
    # zero-padded dense lhsT pairs: chunk c writes row c of the [2, CW]
    # loc/scale PSUM tiles.  wkd_c[:, 4t:4t+2] = loc cols, 4t+2:4t+4 = scale.
    wkd0 = np.zeros((U, 2 * RN * GAMMA), np.float32)
    wkd1 = np.zeros((U, 2 * RN * GAMMA), np.float32)
    wkd0[:, 0 :: 2 * RN] = wk[:, :, 0]          # loc, chunk0 -> row 0
    wkd1[:, RP :: 2 * RN] = wk[:, :, 0]         # loc, chunk1 -> row RP
    wkd0[:, RN :: 2 * RN] = wk[:, :, 1]         # scale, chunk0
    wkd1[:, RN + RP :: 2 * RN] = wk[:, :, 1]    # scale, chunk1
    wb0 = np.broadcast_to(w_all[:, 2 * U][None, :], (RN, GAMMA))
    cb1 = np.broadcast_to(
        (-(np.float32(C_SP) + w_all[:, 2 * U + 1]))[None, :], (RN, GAMMA)
    )  # negated: softplus comes via -ln(sigmoid(-x))

    shared = {
        "r_w": _round_fp32r(gru_rec_kernel),
        "k_w": _round_fp32r(_k_rows(gru_kernel)),
        "k_col": np.ascontiguousarray(
            np.asarray(gru_kernel, np.float32).reshape(3, U).T
        ),
        "wkd0": _round_fp32r(wkd0),
        "wkd1": _round_fp32r(wkd1),
        "wb0": np.ascontiguousarray(wb0, np.float32),
        "cb1": np.ascontiguousarray(cb1, np.float32),
        "gb": gb,
        "h0_z": _round_fp32r(np.zeros((U, BC), np.float32)),
    }
    in_maps = []
    for c in range(N_CORES):
        sl = slice(c * BC, (c + 1) * BC)
        in_maps.append(
            dict(
                shared,
                x_seq=np.ascontiguousarray(xT[:, sl]),
                eps_seq=np.ascontiguousarray(epsT[:, sl]),
            )
        )
    return in_maps, bool(np.any(gb[:, 3] != 0.0))


def _get_nc(with_b1h=False):
    key = ("nc", with_b1h)
    if key not in _CACHE:
        _CACHE[key] = _build_program(with_b1h)
    return _CACHE[key]


def run(inputs_dict, trace=False, trace_kwargs=None):
    in_maps, with_b1h = _host_prep(**inputs_dict)
    nc = _get_nc(with_b1h)
    res = run_bass_kernel_spmd(
        nc, in_maps, list(range(N_CORES)), trace=trace,
        **(trace_kwargs or {}),
    )
    _CACHE["last_results"] = res
    out = np.empty((B_FULL, GAMMA, 2), np.float32)
    for c in range(N_CORES):
        loc = res.results[c]["loc_fm"]                         # [28, 1024]
        sp = res.results[c]["sp_fm"]                           # [28, 1024]
        sc = np.float32(1e-5) - np.float32(OP_SCALE) * sp
        out[c * BC : (c + 1) * BC, :, 0] = loc.T
        out[c * BC : (c + 1) * BC, :, 1] = sc.T
    return out


def kernel(**inputs):
    return run(inputs, trace=bool(os.environ.get("KERNEL_TRACE")))


# revision 27
# speedup vs baseline: 1126.7794x; 1126.7794x over previous
"""Trainium2 Bass kernel for nn_FIB_RNN (GRU encoder + autoregressive
sampling decoder with DenseVariational head).

Contract: kernel(**inputs) takes the FULL unsharded inputs (numpy arrays,
keys as in reference.setup_inputs()) and returns the FULL output
[B, GAMMA, 2] float32.

Strategy: pure data parallelism over the batch dim across 8 NeuronCores
(1024 batch rows per core).  Within a core the GRU state is kept
feature-major [U=128 partitions, batch free] so the recurrent matmul is
lhsT=R_gate[128,128] @ rhs=h[128,512] -> PSUM, and the scalar sequence
input enters as a K=1 outer-product matmul accumulated into the same
PSUM bank.  The tiny DenseVariational weights are sampled on the host
(deterministic given dv_eps) and folded into per-step [128,1] matmuls.

v7 structure (875us baseline -> 743us -> 662us):
 - scheduler realignment: the Tile scheduler's cost model assumed the PE
   warms to 2.4 GHz, but the HAM never grants it on this kernel; pinning
   the modeled clock to 1.2 GHz (KERNEL_SCHED_COLD_PE=1) fixed the ACT
   queue order (tanh was head-of-line blocked behind the other chunk's
   sigmoids) and cut the encoder from 8.1 to 6.9 us/step.
 - dense head on 2 PSUM partitions: zero-padded lhsT pairs accumulate
   chunk c's loc/scale pre-acts into row c*32 of one [33, CW] PSUM tile
   (matmul base partitions must be 0/32/64, and lhsT/rhs must share the
   base — K row replicated to partition 32 for the chunk-1 K@y matmul).
   sigmoid/ln/loc/AMR/y-add each handle both chunks in ONE FD=512
   instruction, cutting ~2us off the serial sampling tail per step.

v3 structure (vs the v1 baseline, 875us -> 743us):
 - bf16 matmul operands + h-state (default KERNEL_MM_DT=bf16): the PE
   streams 2 cols/cycle and the d/e/h2 update chain hits the DVE 2x_1P
   packed mode (379ns vs 614-666ns per op).  Measured rel err 1.30e-2
   vs the 2e-2 gate; KERNEL_MM_DT=f32r is the high-precision fallback.
 - scale output finished on the HOST: the kernel ships ln(g) rows; the
   affine 1e-5 - 0.05*ln(g) is applied in numpy after the gather.
   Removes 56 Identity acts + 56 row DMAs from the decoder loop.
 - decoder emission reordered: next step's R@h matmuls are enqueued
   before the dense head's row ops, so the PE works through the serial
   sampling tail; K@y matmuls issue per chunk right after that chunk's
   y row is ready.
 - loc rows on the DVE (tensor_scalar from PSUM) so the ACT queue is
   free for the sigmoid->ACT_TABLE_LOAD->Ln->reload chain.

Negative results (tried, reverted): PE-warming dummy matmuls (fill HAM
activity window with junk MMs into a scratch PSUM bank): warming IS
real (matmul avg 475->417ns, warm bursts at 215-258ns) but 10 dummies
per step block the in-order PE queue longer than the warming saves
(743us vs 662) - a tuned count ~4-5/step sized to the warm-case idle
gap is the most promising unexplored lead; GpSimd CANNOT access PSUM (BIR
verifier: "GPSIMD Instructions cannot access PSUM") - no pool STT on
matmul outputs; tt/uu in bf16 (encoder STT 2x candidate) pushes sim rel
err 9.1e-3 -> 1.31e-2, projected HW ~1.9e-2 vs the 2e-2 gate - too
thin; e/h2 on the GpSimd engine (Pool TT
ops cost ~1.4us and sit on the recurrent critical chain); tanh as
2*sigmoid(2x)-1 via LN_BWD_DX_ANT (795ns vs 614ns plain sub); a custom
PWP act-table root merging ln into sigmoid_and_others (ctrl-word format
is (cfg<<11)|bucket_base and the merge fits 1536 buckets, but NRT
rejects the NEFF at load - the runtime has its own baked table images).
KERNEL_TABLE_MERGE=1 keeps that experiment reachable.
"""

import json
import os
import shutil
import sys
from contextlib import ExitStack
from pathlib import Path

import numpy as np

for _p in ("/opt/trn_rl_repo", "/root/.axon_site/_ro/trn_rl_repo"):
    if os.path.isdir(_p) and _p not in sys.path:
        sys.path.insert(0, _p)

F32 = None  # set after imports
U = 128                    # rnn units
T_ENC = 48                 # encoder steps
GAMMA = 28                 # decoder outputs (27 sampled feedback steps)
N_CORES = 8
B_FULL = 8192
BC = B_FULL // N_CORES     # 1024 batch rows per core
CW = 512                   # chunk width (PSUM bank = 512 fp32)
NCH = BC // CW             # 2 chunks per core
RP = 32                    # dense-head row stride (matmul base-partition rule)
RN = RP + 1                # dense-head tile partition count (rows 0 and RP live)
C_SP = float(np.log(np.expm1(1.0)))  # softplus^-1(1.0)
Q_SCALE = 0.02
OP_SCALE = 0.05

_MERGE_TABLES = os.environ.get("KERNEL_TABLE_MERGE", "0") == "1"

# ---------------------------------------------------------------------------
# Custom activation-table root: sigmoid_and_others + ln merged.
# ---------------------------------------------------------------------------

_PWP_SRC = (
    "/nix/store/z022hj2nvbm3nwdizlisq4ylc0y7rd6q-python3-3.13.14-env/"
    "lib/python3.13/site-packages/neuronxcc/pwp/pwp_bin_trainium"
)
_DROP_FUNCS = ("tanh", "erf", "arctan")
_LN_EXP_LO, _LN_EXP_HI = -32, 1   # keep ln buckets for input exp in [lo, hi]


def _build_merged_act_root(dst: str) -> str:
    """Create a PWP act root at dst where sigmoid_and_others also serves
    ln.  Ctrl entries are (cfg<<11)|bucket_base words; rebasing = adjust
    the low 11 bits.  Returns path to the new act_info.json."""
    src = Path(_PWP_SRC)
    dstp = Path(dst)
    if (dstp / "act_info.json").exists():
        return str(dstp / "act_info.json")
    dstp.mkdir(parents=True, exist_ok=True)
    info = json.loads((src / "act_info.json").read_text())

    for ent in info["act_func_sets"]:
        if ent["name"] != "sigmoid_and_others":
            for k in ("bkt_bin", "ctrl_bin", "profile_json"):
                if not (dstp / ent[k]).exists():
                    shutil.copy(src / ent[k], dstp / ent[k])
            continue

        prof = json.loads((src / ent["profile_json"]).read_text())
        bkt = np.fromfile(src / ent["bkt_bin"], dtype=np.uint32).reshape(-1, 8)
        ctl = np.fromfile(src / ent["ctrl_bin"], dtype=np.uint32).reshape(-1, 8)
        nlj = json.loads((src / "natural_log.json").read_text())
        nl_bkt = np.fromfile(src / "natural_log_bkt.bin", dtype=np.uint32).reshape(-1, 8)
        nl_ctl = np.fromfile(src / "natural_log_ctrl.bin", dtype=np.uint32).reshape(-1, 8)

        funcs = list(prof["func_to_bkt_start_idx"].keys())
        n_bkt, n_ctl = prof["bkt_entry_cnt"], prof["ctl_entry_cnt"]
        bkt_start = dict(prof["func_to_bkt_start_idx"])
        ctl_start = dict(prof["func_to_ctl_start_idx"])
        order = sorted(funcs, key=lambda f: bkt_start[f])

        def _blk(d, f, total):
            o = sorted(funcs, key=lambda x: d[x])
            i = o.index(f)
            end = d[o[i + 1]] if i + 1 < len(o) else total
            return d[f], end

        keep = [f for f in order if f not in _DROP_FUNCS]
        new_bkt_rows, new_ctl_rows = [], []
        new_bs, new_cs = {}, {}
        meta_by_name = {m["func_name"]: m for m in prof["profile_meta_data"]}
        new_meta = []
        new_exp_bkt, new_exp_ctl = {}, {}
        for f in keep:
            b0, b1 = _blk(bkt_start, f, n_bkt)
            c0, c1 = _blk(ctl_start, f, n_ctl)
            nb, ncs = len(new_bkt_rows), len(new_ctl_rows)
            new_bs[f] = nb
            new_cs[f] = ncs
            new_bkt_rows.extend(bkt[b0:b1])
            blkc = ctl[c0:c1].copy()
            # rebase bucket pointers: low 11 bits
            base = blkc[:, 0] & 0x7FF
            cfg = blkc[:, 0] & ~np.uint32(0x7FF)
            blkc[:, 0] = cfg | ((base - b0 + nb) & 0x7FF)
            new_ctl_rows.extend(blkc)
            eb = prof.get("func_exp_to_bkt_start_idx", {}).get(f, {})
            new_exp_bkt[f] = {
                e: [v - b0 + nb for v in vs] for e, vs in eb.items()
            }
            ec = prof.get("func_exp_to_ctl_start_idx", {}).get(f, {})
            new_exp_ctl[f] = {
                e: [v - c0 + ncs for v in vs] for e, vs in ec.items()
            }
            mm = [m for m in prof["profile_meta_data"]
                  if m["func_name"].rsplit("_", 1)[0] == f]
            new_meta.extend(mm)

        # ---- append trimmed ln from natural_log ----
        ln_exp_bkt = nlj["func_exp_to_bkt_start_idx"]["ln"]
        ln_exp_ctl = nlj["func_exp_to_ctl_start_idx"]["ln"]
        exps = sorted(int(e) for e in ln_exp_bkt)
        keep_exps = [e for e in exps if _LN_EXP_LO <= e <= _LN_EXP_HI]
        lo_bkt = min(ln_exp_bkt[str(e)][0] for e in keep_exps)
        hi_bkt = max(
            (ln_exp_bkt[str(e + 1)][0] if str(e + 1) in ln_exp_bkt else None)
            or nlj["func_to_bkt_start_idx"]["relu"]
            for e in keep_exps
        )
        lo_ctl = min(ln_exp_ctl[str(e)][0] for e in keep_exps)
        hi_ctl = max(
            (ln_exp_ctl[str(e + 1)][0] if str(e + 1) in ln_exp_ctl else None)
            or nlj["func_to_ctl_start_idx"]["relu"]
            for e in keep_exps
        )
        nb, ncs = len(new_bkt_rows), len(new_ctl_rows)
        assert nb + (hi_bkt - lo_bkt) <= 1536, (nb, hi_bkt - lo_bkt)
        new_bs["ln"] = nb
        new_cs["ln"] = ncs
        new_bkt_rows.extend(nl_bkt[lo_bkt:hi_bkt])
        blkc = nl_ctl[lo_ctl:hi_ctl].copy()
        base = blkc[:, 0] & 0x7FF
        cfg = blkc[:, 0] & ~np.uint32(0x7FF)
        blkc[:, 0] = cfg | ((base - lo_bkt + nb) & 0x7FF)
        new_ctl_rows.extend(blkc)
        new_exp_bkt["ln"] = {
            str(e): [v - lo_bkt + nb for v in ln_exp_bkt[str(e)]]
            for e in keep_exps
        }
        new_exp_ctl["ln"] = {
            str(e): [v - lo_ctl + ncs for v in ln_exp_ctl[str(e)]]
            for e in keep_exps
        }
        ln_meta = [m for m in nlj["profile_meta_data"]
                   if m["func_name"].startswith("ln")]
        assert len(ln_meta) == 1
        lm = dict(ln_meta[0])
        # remap ln's control bases (ctl indices) by the ctl shift
        for key in ("pwl_control_base_pos", "pwl_control_base_neg",
                    "pos_small_signal_pwl_control", "neg_small_signal_pwl_control",
                    "pos_large_signal_pwl_control", "neg_large_signal_pwl_control"):
            if key in lm and isinstance(lm[key], int):
                lm[key] = lm[key] - lo_ctl + ncs
        # clamp bounds to the kept bucket range: [2^lo_exp, 2^(hi_exp+1))
        lm["lower_bound"] = int(
            np.float32(2.0 ** _LN_EXP_LO).view(np.uint32)
        )
        lm["upper_bound"] = int(
            np.nextafter(np.float32(2.0 ** (_LN_EXP_HI + 1)),
                         np.float32(0.0)).view(np.uint32)
        )
        new_meta.append(lm)

        bkt_arr = np.vstack(new_bkt_rows).astype(np.uint32)
        ctl_arr = np.vstack(new_ctl_rows).astype(np.uint32)
        bkt_arr.tofile(dstp / ent["bkt_bin"])
        ctl_arr.tofile(dstp / ent["ctrl_bin"])
        prof2 = dict(prof)
        prof2["bkt_entry_cnt"] = int(bkt_arr.shape[0])
        prof2["ctl_entry_cnt"] = int(ctl_arr.shape[0])
        prof2["func_to_bkt_start_idx"] = new_bs
        prof2["func_to_ctl_start_idx"] = new_cs
        prof2["func_exp_to_bkt_start_idx"] = new_exp_bkt
        prof2["func_exp_to_ctl_start_idx"] = new_exp_ctl
        prof2["profile_meta_data"] = new_meta
        (dstp / ent["profile_json"]).write_text(json.dumps(prof2))
        act = {f: n for f, n in ent["act"].items() if f not in _DROP_FUNCS}
        act["ln"] = 40
        ent["act"] = act

    (dstp / "act_info.json").write_text(json.dumps(info))
    return str(dstp / "act_info.json")


if _MERGE_TABLES and os.path.isdir(_PWP_SRC):
    _root = _build_merged_act_root("/tmp/kernel_act_root_v1")
    os.environ["BASS_ACT_ROOT_JSON_PATH"] = _root
else:
    _MERGE_TABLES = False

import concourse.bass as bass
import concourse.tile as tile
from concourse import bacc, mybir
from concourse.bass_utils import run_bass_kernel_spmd
from concourse.dve_ops import AFFINE_MUL_REDUCE, LN_BWD_DX_ANT

F32 = mybir.dt.float32
AF = mybir.ActivationFunctionType
ALU = mybir.AluOpType

# The Tile scheduler's cost model assumes the PE ramps to 2.4 GHz after
# 3us of continuous work; on this kernel the HAM never grants the warm
# clock (measured: all matmuls at ~1.2 GHz).  Pinning the modeled warm
# clock to 1.2 GHz realigns the scheduler's readiness estimates with
# reality (pure scheduling hint; no semantic change).
if os.environ.get("KERNEL_SCHED_COLD_PE", "1") == "1":
    from concourse.hw_specs import TRN2Spec as _Spec

    _Spec.PE_CYCLE = 1e9 / 1.2e9

if _MERGE_TABLES:
    # keep the bass-side view of the table sets consistent with the
    # custom act root (the returned dict is cached; mutate in place).
    from concourse.hw_specs import get_activation_tables

    _tabs = get_activation_tables("gen3")
    _sig = _tabs["sigmoid_and_others"]
    _sig.add(AF.Ln)
    _sig.discard(AF.Tanh)
    for _af in ("Erf", "Arctan"):
        if hasattr(AF, _af):
            _sig.discard(getattr(AF, _af))

# matmul operand dtype.  float32r (e8m11-rounded fp32) streams 1 col/cycle
# vs 4 cycles/col for full fp32; bfloat16 streams 2 cols/cycle.
_MM_MODE = os.environ.get("KERNEL_MM_DT", "bf16")
RD = {"f32r": mybir.dt.float32r, "bf16": mybir.dt.bfloat16, "f32": F32}[_MM_MODE]
RD16 = mybir.dt.bfloat16 if _MM_MODE == "bf16" else F32

_CACHE = {}


def _round_fp32r(a):
    a = np.ascontiguousarray(a, np.float32)
    if _MM_MODE == "f32":
        return a
    if _MM_MODE == "bf16":
        import ml_dtypes
        return np.ascontiguousarray(a.astype(ml_dtypes.bfloat16))
    bits = a.view(np.uint32)
    out = ((bits.astype(np.uint64) + 0x800) & 0xFFFFF000).astype(np.uint32)
    return out.view(np.float32)


def _build_program(with_b1h):
    nc = bacc.Bacc("TRN2", target_bir_lowering=False, debug=False)

    x_seq = nc.dram_tensor("x_seq", [T_ENC, BC], RD, kind="ExternalInput").ap()
    eps_seq = nc.dram_tensor("eps_seq", [GAMMA - 1, BC], F32, kind="ExternalInput").ap()
    r_w = nc.dram_tensor("r_w", [U, 3 * U], RD, kind="ExternalInput").ap()
    k_w = nc.dram_tensor("k_w", [RN, 3 * U], RD, kind="ExternalInput").ap()
    k_col = nc.dram_tensor("k_col", [U, 3], F32, kind="ExternalInput").ap()
    wkd0 = nc.dram_tensor("wkd0", [U, 2 * RN * GAMMA], RD, kind="ExternalInput").ap()
    wkd1 = nc.dram_tensor("wkd1", [U, 2 * RN * GAMMA], RD, kind="ExternalInput").ap()
    wb0 = nc.dram_tensor("wb0", [RN, GAMMA], F32, kind="ExternalInput").ap()
    cb1 = nc.dram_tensor("cb1", [RN, GAMMA], F32, kind="ExternalInput").ap()
    gb = nc.dram_tensor("gb", [U, 4], F32, kind="ExternalInput").ap()
    h0_z = nc.dram_tensor("h0_z", [U, BC], RD, kind="ExternalInput").ap()
    loc_fm = nc.dram_tensor("loc_fm", [GAMMA, BC], F32, kind="ExternalOutput").ap()
    sp_fm = nc.dram_tensor("sp_fm", [GAMMA, BC], F32, kind="ExternalOutput").ap()

    with tile.TileContext(nc) as tc, ExitStack() as es:
        consts = es.enter_context(tc.tile_pool(name="consts", bufs=1))
        R = consts.tile([U, 3 * U], RD)
        K = consts.tile([RN, 3 * U], RD)
        KC = consts.tile([U, 3], F32)
        WKD0 = consts.tile([U, 2 * RN * GAMMA], RD)
        WKD1 = consts.tile([U, 2 * RN * GAMMA], RD)
        WB0 = consts.tile([RN, GAMMA], F32)
        CB1 = consts.tile([RN, GAMMA], F32)
        GB = consts.tile([U, 4], F32)
        EPT = consts.tile([RN, CW], F32)
        nc.vector.memset(EPT[:], 0.0)
        nc.sync.dma_start(R[:], r_w[:])
        nc.sync.dma_start(K[:], k_w[:])
        nc.sync.dma_start(KC[:], k_col[:])
        nc.sync.dma_start(WKD0[:], wkd0[:])
        nc.sync.dma_start(WKD1[:], wkd1[:])
        nc.sync.dma_start(WB0[:], wb0[:])
        nc.sync.dma_start(CB1[:], cb1[:])
        nc.sync.dma_start(GB[:], gb[:])

        hpool = es.enter_context(tc.tile_pool(name="h", bufs=4))
        gates = es.enter_context(tc.tile_pool(name="gates", bufs=3))
        samp = es.enter_context(tc.tile_pool(name="samp", bufs=2))
        stage = es.enter_context(tc.tile_pool(name="stage", bufs=5))
        ps_g = es.enter_context(tc.tile_pool(
            name="psg", bufs=int(os.environ.get("KERNEL_PS_BUFS", "7")),
            space="PSUM"))
        ps_d = es.enter_context(tc.tile_pool(name="psd", bufs=1, space="PSUM"))
        DUM = ps_d.tile([U, 128], F32, tag="dum")
        n_dum_e = int(os.environ.get("KERNEL_DUMMY_ENC", "8"))
        n_dum_d = int(os.environ.get("KERNEL_DUMMY_DEC", "14"))

        def pe_warm(n):
            """Dependency-free 128-col junk matmuls into a scratch PSUM
            bank.  The HAM clock-gate only grants the 2.4 GHz PE clock
            while its activity window stays busy; these fill PE-idle gaps.
            128-col quanta (~107ns cold / ~55ns warm) keep the in-order
            queue block far smaller than the h2/y wait they fill (the
            v9 failure was 384-col x10 = 3.2us of blocking)."""
            for _ in range(n):
                nc.tensor.matmul(DUM[:], R[:, z3], R[:, 0:128],
                                 start=True, stop=True)

        hT = hpool.tile([U, BC], RD, tag="h")
        nc.sync.dma_start(hT[:], h0_z[:])
        hbox = [hT, None]  # [current h, next h being written]

        z3 = bass.ts(0, U)
        r3 = bass.ts(1, U)
        h3 = bass.ts(2, U)

        def gru_elem(c, psr, psh, psz, psx, xb):
            """Post-matmul element chain for chunk c ([U,CW] PSUM tiles).
            Either xb (encoder broadcast tile; x*K_h enters via STT) or
            psx (decoder; K_h@y PSUM)."""
            cs = bass.ts(c, CW)
            hc = hbox[0][:, cs]
            # r = sigmoid(rho + br)                     (GB1 = br)
            r_ = gates.tile([U, CW], RD16, tag=f"r_{c}")
            nc.scalar.activation(r_[:], psr[:], AF.Sigmoid,
                                 bias=GB[:, 1:2], scale=1.0)
            hrec = psh[:]
            if with_b1h:
                hb = gates.tile([U, CW], F32, tag=f"hb_{c}")
                nc.vector.tensor_scalar(
                    hb[:], psh[:], GB[:, 3:4], None, op0=ALU.add
                )
                hrec = hb[:]
            tt = gates.tile([U, CW], F32, tag=f"t_{c}")
            nc.vector.tensor_mul(tt[:], r_[:], hrec)
            uu = gates.tile([U, CW], F32, tag=f"u_{c}")
            if xb is not None:
                nc.vector.scalar_tensor_tensor(
                    uu[:], xb[:, cs], KC[:, 2:3], tt[:],
                    op0=ALU.mult, op1=ALU.add,
                )
            else:
                nc.vector.tensor_add(uu[:], tt[:], psx[:])
            hh = gates.tile([U, CW], RD16, tag=f"hh_{c}")
            nc.scalar.activation(hh[:], uu[:], AF.Tanh, bias=GB[:, 2:3],
                                 scale=1.0)
            # u1 = 1-z = sigmoid(-(zeta + bz))  (GB0 = -bz).  Emitted after
            # the tanh: u1 is needed only by e, and an earlier emission lets
            # the scheduler queue-block the tanh behind the other chunk's
            # sigmoids on the in-order ACT engine.
            u1 = gates.tile([U, CW], RD16, tag=f"u1_{c}")
            nc.scalar.activation(u1[:], psz[:], AF.Sigmoid,
                                 bias=GB[:, 0:1], scale=-1.0)
            # h' = h + (1-z)*(hh - h): three consecutive DVE ops (bf16
            # operands hit the 2x_1P packed mode)
            d = gates.tile([U, CW], RD16, tag=f"d_{c}")
            nc.vector.tensor_sub(d[:], hh[:], hc)
            e = gates.tile([U, CW], RD16, tag=f"e_{c}")
            nc.vector.tensor_mul(e[:], u1[:], d[:])
            nc.vector.tensor_add(hbox[1][:, cs], hc, e[:])

        # ---- encoder: 48 GRU steps ----
        for t in range(T_ENC):
            xb = stage.tile([U, BC], RD, tag="xb")
            nc.sync.dma_start(xb[:], x_seq[t : t + 1, :].partition_broadcast(U))
            hT = hbox[0]
            h_next = hpool.tile([U, BC], RD, tag="h")
            hbox[1] = h_next
            for c in range(NCH):
                cs = bass.ts(c, CW)
                x_row = xb[0:1, cs]
                psr = ps_g.tile([U, CW], F32, tag="ps")
                psh = ps_g.tile([U, CW], F32, tag="ps")
                psz = ps_g.tile([U, CW], F32, tag="ps")
                nc.tensor.matmul(psr[:], K[0:1, r3], x_row, start=True, stop=False)
                nc.tensor.matmul(psr[:], R[:, r3], hT[:, cs], start=False, stop=True)
                nc.tensor.matmul(psh[:], R[:, h3], hT[:, cs], start=True, stop=True)
                nc.tensor.matmul(psz[:], K[0:1, z3], x_row, start=True, stop=False)
                nc.tensor.matmul(psz[:], R[:, z3], hT[:, cs], start=False, stop=True)
                gru_elem(c, psr, psh, psz, None, xb)
            pe_warm(n_dum_e)
            hbox[0] = hbox[1]

        # ---- decoder ----
        def dense_mms(t):
            """Dense head matmuls for step t.  Both chunks land in one
            [2, CW] PSUM tile (row c = chunk c) via zero-padded lhsT
            columns accumulated across two matmuls, so every downstream
            row op handles both chunks in a single instruction."""
            hT = hbox[0]
            psl = ps_g.tile([RN, CW], F32, tag="ps")
            pss = ps_g.tile([RN, CW], F32, tag="ps")
            for c, WKD in ((0, WKD0), (1, WKD1)):
                cs = bass.ts(c, CW)
                o = 2 * RN * t
                nc.tensor.matmul(psl[:], WKD[:, o : o + RN], hT[:, cs],
                                 start=(c == 0), stop=(c == 1))
                nc.tensor.matmul(pss[:], WKD[:, o + RN : o + 2 * RN],
                                 hT[:, cs], start=(c == 0), stop=(c == 1))
            return psl, pss

        def head_rows(t, ps):
            """sigmoid+ln+loc rows for step t ([2, CW]: row c = chunk c);
            DMA out; returns (loc, sp)."""
            psl, pss = ps
            g = samp.tile([RN, CW], F32, tag="g")
            loc = samp.tile([RN, CW], F32, tag="loc")
            # g = sigmoid(-(s + C + wb1))   (CB1 = -(C+wb1))
            nc.scalar.activation(
                g[:], pss[:], AF.Sigmoid, bias=CB1[:, t : t + 1],
                scale=-1.0,
            )
            # loc = h@W0 + wb0   (DVE: keeps the ACT queue free for the
            # table-load + Ln chain)
            nc.vector.tensor_scalar(
                loc[:], psl[:], WB0[:, t : t + 1], None, op0=ALU.add,
            )
            # sp = ln(g) = -softplus  (host applies 1e-5 - 0.05*sp)
            sp = samp.tile([RN, CW], F32, tag="sp")
            nc.scalar.activation(sp[:], g[:], AF.Ln, bias=0.0, scale=1.0)
            for c in range(NCH):
                cs = bass.ts(c, CW)
                rr = c * RP
                nc.sync.dma_start(loc_fm[t : t + 1, cs], loc[rr : rr + 1, :])
                nc.sync.dma_start(sp_fm[t : t + 1, cs], sp[rr : rr + 1, :])
            return loc, sp

        ps = dense_mms(0)
        for t in range(1, GAMMA):
            # next step's recurrent matmuls first: PE works through the tail
            hT = hbox[0]
            gps = []
            for c in range(NCH):
                cs = bass.ts(c, CW)
                psr = ps_g.tile([U, CW], F32, tag="ps")
                psh = ps_g.tile([U, CW], F32, tag="ps")
                psz = ps_g.tile([U, CW], F32, tag="ps")
                psx = ps_g.tile([U, CW], F32, tag="ps")
                nc.tensor.matmul(psr[:], R[:, r3], hT[:, cs], start=True,
                                 stop=False)
                nc.tensor.matmul(psh[:], R[:, h3], hT[:, cs], start=True,
                                 stop=True)
                nc.tensor.matmul(psz[:], R[:, z3], hT[:, cs], start=True,
                                 stop=False)
                gps.append((psr, psh, psz, psx))
            # before K@y in the queue, which waits on y anyway: these can
            # never delay real work, only bridge the sampling-tail idle
            pe_warm(n_dum_d)
            loc, sp = head_rows(t - 1, ps)
            # sample: y = loc + (1e-5 - 0.05*sp) * eps, rows {0, RP}
            ep = EPT
            for c in range(NCH):
                cs = bass.ts(c, CW)
                rr = c * RP
                nc.sync.dma_start(ep[rr : rr + 1, :], eps_seq[t - 1 : t, cs])
            m = samp.tile([RN, CW], F32, tag="m")
            nc.vector._custom_dve(
                AFFINE_MUL_REDUCE, out=m[:], in0=sp[:], in1=ep[:],
                s0=-OP_SCALE, s1=1e-5,
            )
            y = samp.tile([RN, CW], RD, tag="y")
            nc.vector.tensor_add(y[:], m[:], loc[:])
            h_next = hpool.tile([U, BC], RD, tag="h")
            hbox[1] = h_next
            for c in range(NCH):
                psr, psh, psz, psx = gps[c]
                rr = c * RP
                y_row = y[rr : rr + 1, :]
                Kc = K[rr : rr + 1, :]
                nc.tensor.matmul(psr[:], Kc[:, r3], y_row, start=False,
                                 stop=True)
                nc.tensor.matmul(psx[:], Kc[:, h3], y_row, start=True,
                                 stop=True)
                nc.tensor.matmul(psz[:], Kc[:, z3], y_row, start=False,
                                 stop=True)
                gru_elem(c, psr, psh, psz, psx, None)
            hbox[0] = hbox[1]
            ps = dense_mms(t)
        head_rows(GAMMA - 1, ps)

    nc.compile()
    return nc


def _k_rows(gru_kernel):
    k = np.zeros((RN, 3 * U), np.float32)
    k[0] = np.asarray(gru_kernel, np.float32)[0]
    k[RP] = k[0]
    return k


def _host_prep(inputs, gru_kernel, gru_rec_kernel, gru_bias, dv_loc, dv_rho,
               dv_eps, samp_eps):
    inputs = np.asarray(inputs, np.float32)
    B = inputs.shape[0]
    assert B == B_FULL, f"kernel compiled for B={B_FULL}, got {B}"
    xT = _round_fp32r(inputs[:, :T_ENC, 0].T)                  # [48, B]
    epsT = np.ascontiguousarray(np.asarray(samp_eps, np.float32)[:, :, 0])

    gru_bias = np.asarray(gru_bias, np.float32)
    b0, b1 = gru_bias[0], gru_bias[1]
    gb = np.zeros((U, 4), np.float32)
    gb[:, 0] = -(b0[0:U] + b1[0:U])
    gb[:, 1] = b0[U : 2 * U] + b1[U : 2 * U]
    gb[:, 2] = b0[2 * U : 3 * U]
    gb[:, 3] = b1[2 * U : 3 * U]

    dv_loc = np.asarray(dv_loc, np.float32)
    dv_rho = np.asarray(dv_rho, np.float32)
    dv_eps = np.asarray(dv_eps, np.float32)
    scale_q = np.float32(1e-5) + np.float32(Q_SCALE) * np.logaddexp(
        np.float32(C_SP) + dv_rho, np.float32(0.0), dtype=np.float32
    )
    w_all = dv_loc[None, :] + scale_q[None, :] * dv_eps        # [28, 258]
    wk = w_all[:, : 2 * U].reshape(GAMMA, U, 2).transpose(1, 0, 2)  # [U,28,2]
    # zero-padded dense lhsT pairs: chunk c writes row c of the [2, CW]
    # loc/scale PSUM tiles.  wkd_c[:, 4t:4t+2] = loc cols, 4t+2:4t+4 = scale.
    wkd0 = np.zeros((U, 2 * RN * GAMMA), np.float32)
    wkd1 = np.zeros((U, 2 * RN * GAMMA), np.float32)
    wkd0[:, 0 :: 2 * RN] = wk[:, :, 0]          # loc, chunk0 -> row 0
    wkd1[:, RP :: 2 * RN] = wk[:, :, 0]         # loc, chunk1 -> row RP
    wkd0[:, RN :: 2 * RN] = wk[:, :, 1]         # scale, chunk0
    wkd1[:, RN + RP :: 2 * RN] = wk[:, :, 1]    # scale, chunk1
    wb0 = np.broadcast_to(w_all[:, 2 * U][None, :], (RN, GAMMA))
    cb1 = np.broadcast_to(
        (-(np.float32(C_SP) + w_all[:, 2 * U + 1]))[None, :], (RN, GAMMA)
    )  # negated: softplus comes via -ln(sigmoid(-x))

    shared = {
        "r_w": _round_fp32r(gru_rec_kernel),
        "k_w": _round_fp32r(_k_rows(gru_kernel)),
        "k_col": np.ascontiguousarray(
            np.asarray(gru_kernel, np.float32).reshape(3, U).T
        ),
        "wkd0": _round_fp32r(wkd0),
        "wkd1": _round_fp32r(wkd1),
        "wb0": np.ascontiguousarray(wb0, np.float32),
        "cb1": np.ascontiguousarray(cb1, np.float32),
        "gb": gb,
        "h0_z": _round_fp32r(np.zeros((U, BC), np.float32)),
    }
    in_maps = []
    for c in range(N_CORES):
        sl = slice(c * BC, (c + 1) * BC)
        in_maps.append(
            dict(
                shared,
                x_seq=np.ascontiguousarray(xT[:, sl]),
                eps_seq=np.ascontiguousarray(epsT[:, sl]),
            )
        )
    return in_maps, bool(np.any(gb[:, 3] != 0.0))


def _get_nc(with_b1h=False):
    key = ("nc", with_b1h)
    if key not in _CACHE:
        _CACHE[key] = _build_program(with_b1h)
    return _CACHE[key]


def run(inputs_dict, trace=False, trace_kwargs=None):
    in_maps, with_b1h = _host_prep(**inputs_dict)
    nc = _get_nc(with_b1h)
    res = run_bass_kernel_spmd(
        nc, in_maps, list(range(N_CORES)), trace=trace,
        **(trace_kwargs or {}),
    )
    _CACHE["last_results"] = res
    out = np.empty((B_FULL, GAMMA, 2), np.float32)
    for c in range(N_CORES):
        loc = res.results[c]["loc_fm"]                         # [28, 1024]
        sp = res.results[c]["sp_fm"]                           # [28, 1024]
        sc = np.float32(1e-5) - np.float32(OP_SCALE) * sp
        out[c * BC : (c + 1) * BC, :, 0] = loc.T
        out[c * BC : (c + 1) * BC, :, 1] = sc.T
    return out


def kernel(**inputs):
    return run(inputs, trace=bool(os.environ.get("KERNEL_TRACE")))


# revision 28
# speedup vs baseline: 1134.6255x; 1.0070x over previous
"""Trainium2 Bass kernel for nn_FIB_RNN (GRU encoder + autoregressive
sampling decoder with DenseVariational head).

Contract: kernel(**inputs) takes the FULL unsharded inputs (numpy arrays,
keys as in reference.setup_inputs()) and returns the FULL output
[B, GAMMA, 2] float32.

Strategy: pure data parallelism over the batch dim across 8 NeuronCores
(1024 batch rows per core).  Within a core the GRU state is kept
feature-major [U=128 partitions, batch free] so the recurrent matmul is
lhsT=R_gate[128,128] @ rhs=h[128,512] -> PSUM, and the scalar sequence
input enters as a K=1 outer-product matmul accumulated into the same
PSUM bank.  The tiny DenseVariational weights are sampled on the host
(deterministic given dv_eps) and folded into per-step [128,1] matmuls.

v7 structure (875us baseline -> 743us -> 662us):
 - scheduler realignment: the Tile scheduler's cost model assumed the PE
   warms to 2.4 GHz, but the HAM never grants it on this kernel; pinning
   the modeled clock to 1.2 GHz (KERNEL_SCHED_COLD_PE=1) fixed the ACT
   queue order (tanh was head-of-line blocked behind the other chunk's
   sigmoids) and cut the encoder from 8.1 to 6.9 us/step.
 - dense head on 2 PSUM partitions: zero-padded lhsT pairs accumulate
   chunk c's loc/scale pre-acts into row c*32 of one [33, CW] PSUM tile
   (matmul base partitions must be 0/32/64, and lhsT/rhs must share the
   base — K row replicated to partition 32 for the chunk-1 K@y matmul).
   sigmoid/ln/loc/AMR/y-add each handle both chunks in ONE FD=512
   instruction, cutting ~2us off the serial sampling tail per step.

v3 structure (vs the v1 baseline, 875us -> 743us):
 - bf16 matmul operands + h-state (default KERNEL_MM_DT=bf16): the PE
   streams 2 cols/cycle and the d/e/h2 update chain hits the DVE 2x_1P
   packed mode (379ns vs 614-666ns per op).  Measured rel err 1.30e-2
   vs the 2e-2 gate; KERNEL_MM_DT=f32r is the high-precision fallback.
 - scale output finished on the HOST: the kernel ships ln(g) rows; the
   affine 1e-5 - 0.05*ln(g) is applied in numpy after the gather.
   Removes 56 Identity acts + 56 row DMAs from the decoder loop.
 - decoder emission reordered: next step's R@h matmuls are enqueued
   before the dense head's row ops, so the PE works through the serial
   sampling tail; K@y matmuls issue per chunk right after that chunk's
   y row is ready.
 - loc rows on the DVE (tensor_scalar from PSUM) so the ACT queue is
   free for the sigmoid->ACT_TABLE_LOAD->Ln->reload chain.

PE-warming dummy matmuls (KERNEL_DUMMY_ENC/DEC): warming is real
(matmul avg 475->417ns, warm bursts at 215-258ns) but oversized dummies
block the in-order PE queue (10x384-col = 743us regression); the tuned
128-col variant (8 enc / 14 dec) is roughly neutral-positive (661us) -
the HAM duty still does not sustain 2.4 GHz.  Other negative results: GpSimd CANNOT access PSUM (BIR
verifier: "GPSIMD Instructions cannot access PSUM") - no pool STT on
matmul outputs; tt/uu in bf16 (encoder STT 2x candidate) pushes sim rel
err 9.1e-3 -> 1.31e-2, projected HW ~1.9e-2 vs the 2e-2 gate - too
thin; e/h2 on the GpSimd engine (Pool TT
ops cost ~1.4us and sit on the recurrent critical chain); tanh as
2*sigmoid(2x)-1 via LN_BWD_DX_ANT (795ns vs 614ns plain sub); a custom
PWP act-table root merging ln into sigmoid_and_others (ctrl-word format
is (cfg<<11)|bucket_base and the merge fits 1536 buckets, but NRT
rejects the NEFF at load - the runtime has its own baked table images).
KERNEL_TABLE_MERGE=1 keeps that experiment reachable.
"""

import json
import os
import shutil
import sys
from contextlib import ExitStack
from pathlib import Path

import numpy as np

for _p in ("/opt/trn_rl_repo", "/root/.axon_site/_ro/trn_rl_repo"):
    if os.path.isdir(_p) and _p not in sys.path:
        sys.path.insert(0, _p)

F32 = None  # set after imports
U = 128                    # rnn units
T_ENC = 48                 # encoder steps
GAMMA = 28                 # decoder outputs (27 sampled feedback steps)
N_CORES = 8
B_FULL = 8192
BC = B_FULL // N_CORES     # 1024 batch rows per core
CW = 512                   # chunk width (PSUM bank = 512 fp32)
NCH = BC // CW             # 2 chunks per core
RP = 32                    # dense-head row stride (matmul base-partition rule)
RN = RP + 1                # dense-head tile partition count (rows 0 and RP live)
C_SP = float(np.log(np.expm1(1.0)))  # softplus^-1(1.0)
Q_SCALE = 0.02
OP_SCALE = 0.05

_MERGE_TABLES = os.environ.get("KERNEL_TABLE_MERGE", "0") == "1"

# ---------------------------------------------------------------------------
# Custom activation-table root: sigmoid_and_others + ln merged.
# ---------------------------------------------------------------------------

_PWP_SRC = (
    "/nix/store/z022hj2nvbm3nwdizlisq4ylc0y7rd6q-python3-3.13.14-env/"
    "lib/python3.13/site-packages/neuronxcc/pwp/pwp_bin_trainium"
)
_DROP_FUNCS = ("tanh", "erf", "arctan")
_LN_EXP_LO, _LN_EXP_HI = -32, 1   # keep ln buckets for input exp in [lo, hi]


def _build_merged_act_root(dst: str) -> str:
    """Create a PWP act root at dst where sigmoid_and_others also serves
    ln.  Ctrl entries are (cfg<<11)|bucket_base words; rebasing = adjust
    the low 11 bits.  Returns path to the new act_info.json."""
    src = Path(_PWP_SRC)
    dstp = Path(dst)
    if (dstp / "act_info.json").exists():
        return str(dstp / "act_info.json")
    dstp.mkdir(parents=True, exist_ok=True)
    info = json.loads((src / "act_info.json").read_text())

    for ent in info["act_func_sets"]:
        if ent["name"] != "sigmoid_and_others":
            for k in ("bkt_bin", "ctrl_bin", "profile_json"):
                if not (dstp / ent[k]).exists():
                    shutil.copy(src / ent[k], dstp / ent[k])
            continue

        prof = json.loads((src / ent["profile_json"]).read_text())
        bkt = np.fromfile(src / ent["bkt_bin"], dtype=np.uint32).reshape(-1, 8)
        ctl = np.fromfile(src / ent["ctrl_bin"], dtype=np.uint32).reshape(-1, 8)
        nlj = json.loads((src / "natural_log.json").read_text())
        nl_bkt = np.fromfile(src / "natural_log_bkt.bin", dtype=np.uint32).reshape(-1, 8)
        nl_ctl = np.fromfile(src / "natural_log_ctrl.bin", dtype=np.uint32).reshape(-1, 8)

        funcs = list(prof["func_to_bkt_start_idx"].keys())
        n_bkt, n_ctl = prof["bkt_entry_cnt"], prof["ctl_entry_cnt"]
        bkt_start = dict(prof["func_to_bkt_start_idx"])
        ctl_start = dict(prof["func_to_ctl_start_idx"])
        order = sorted(funcs, key=lambda f: bkt_start[f])

        def _blk(d, f, total):
            o = sorted(funcs, key=lambda x: d[x])
            i = o.index(f)
            end = d[o[i + 1]] if i + 1 < len(o) else total
            return d[f], end

        keep = [f for f in order if f not in _DROP_FUNCS]
        new_bkt_rows, new_ctl_rows = [], []
        new_bs, new_cs = {}, {}
        meta_by_name = {m["func_name"]: m for m in prof["profile_meta_data"]}
        new_meta = []
        new_exp_bkt, new_exp_ctl = {}, {}
        for f in keep:
            b0, b1 = _blk(bkt_start, f, n_bkt)
            c0, c1 = _blk(ctl_start, f, n_ctl)
            nb, ncs = len(new_bkt_rows), len(new_ctl_rows)
            new_bs[f] = nb
            new_cs[f] = ncs
            new_bkt_rows.extend(bkt[b0:b1])
            blkc = ctl[c0:c1].copy()
            # rebase bucket pointers: low 11 bits
            base = blkc[:, 0] & 0x7FF
            cfg = blkc[:, 0] & ~np.uint32(0x7FF)
            blkc[:, 0] = cfg | ((base - b0 + nb) & 0x7FF)
            new_ctl_rows.extend(blkc)
            eb = prof.get("func_exp_to_bkt_start_idx", {}).get(f, {})
            new_exp_bkt[f] = {
                e: [v - b0 + nb for v in vs] for e, vs in eb.items()
            }
            ec = prof.get("func_exp_to_ctl_start_idx", {}).get(f, {})
            new_exp_ctl[f] = {
                e: [v - c0 + ncs for v in vs] for e, vs in ec.items()
            }
            mm = [m for m in prof["profile_meta_data"]
                  if m["func_name"].rsplit("_", 1)[0] == f]
            new_meta.extend(mm)

        # ---- append trimmed ln from natural_log ----
        ln_exp_bkt = nlj["func_exp_to_bkt_start_idx"]["ln"]
        ln_exp_ctl = nlj["func_exp_to_ctl_start_idx"]["ln"]
        exps = sorted(int(e) for e in ln_exp_bkt)
        keep_exps = [e for e in exps if _LN_EXP_LO <= e <= _LN_EXP_HI]
        lo_bkt = min(ln_exp_bkt[str(e)][0] for e in keep_exps)
        hi_bkt = max(
            (ln_exp_bkt[str(e + 1)][0] if str(e + 1) in ln_exp_bkt else None)
            or nlj["func_to_bkt_start_idx"]["relu"]
            for e in keep_exps
        )
        lo_ctl = min(ln_exp_ctl[str(e)][0] for e in keep_exps)
        hi_ctl = max(
            (ln_exp_ctl[str(e + 1)][0] if str(e + 1) in ln_exp_ctl else None)
            or nlj["func_to_ctl_start_idx"]["relu"]
            for e in keep_exps
        )
        nb, ncs = len(new_bkt_rows), len(new_ctl_rows)
        assert nb + (hi_bkt - lo_bkt) <= 1536, (nb, hi_bkt - lo_bkt)
        new_bs["ln"] = nb
        new_cs["ln"] = ncs
        new_bkt_rows.extend(nl_bkt[lo_bkt:hi_bkt])
        blkc = nl_ctl[lo_ctl:hi_ctl].copy()
        base = blkc[:, 0] & 0x7FF
        cfg = blkc[:, 0] & ~np.uint32(0x7FF)
        blkc[:, 0] = cfg | ((base - lo_bkt + nb) & 0x7FF)
        new_ctl_rows.extend(blkc)
        new_exp_bkt["ln"] = {
            str(e): [v - lo_bkt + nb for v in ln_exp_bkt[str(e)]]
            for e in keep_exps
        }
        new_exp_ctl["ln"] = {
            str(e): [v - lo_ctl + ncs for v in ln_exp_ctl[str(e)]]
            for e in keep_exps
        }
        ln_meta = [m for m in nlj["profile_meta_data"]
                   if m["func_name"].startswith("ln")]
        assert len(ln_meta) == 1
        lm = dict(ln_meta[0])
        # remap ln's control bases (ctl indices) by the ctl shift
        for key in ("pwl_control_base_pos", "pwl_control_base_neg",
                    "pos_small_signal_pwl_control", "neg_small_signal_pwl_control",
                    "pos_large_signal_pwl_control", "neg_large_signal_pwl_control"):
            if key in lm and isinstance(lm[key], int):
                lm[key] = lm[key] - lo_ctl + ncs
        # clamp bounds to the kept bucket range: [2^lo_exp, 2^(hi_exp+1))
        lm["lower_bound"] = int(
            np.float32(2.0 ** _LN_EXP_LO).view(np.uint32)
        )
        lm["upper_bound"] = int(
            np.nextafter(np.float32(2.0 ** (_LN_EXP_HI + 1)),
                         np.float32(0.0)).view(np.uint32)
        )
        new_meta.append(lm)

        bkt_arr = np.vstack(new_bkt_rows).astype(np.uint32)
        ctl_arr = np.vstack(new_ctl_rows).astype(np.uint32)
        bkt_arr.tofile(dstp / ent["bkt_bin"])
        ctl_arr.tofile(dstp / ent["ctrl_bin"])
        prof2 = dict(prof)
        prof2["bkt_entry_cnt"] = int(bkt_arr.shape[0])
        prof2["ctl_entry_cnt"] = int(ctl_arr.shape[0])
        prof2["func_to_bkt_start_idx"] = new_bs
        prof2["func_to_ctl_start_idx"] = new_cs
        prof2["func_exp_to_bkt_start_idx"] = new_exp_bkt
        prof2["func_exp_to_ctl_start_idx"] = new_exp_ctl
        prof2["profile_meta_data"] = new_meta
        (dstp / ent["profile_json"]).write_text(json.dumps(prof2))
        act = {f: n for f, n in ent["act"].items() if f not in _DROP_FUNCS}
        act["ln"] = 40
        ent["act"] = act

    (dstp / "act_info.json").write_text(json.dumps(info))
    return str(dstp / "act_info.json")


if _MERGE_TABLES and os.path.isdir(_PWP_SRC):
    _root = _build_merged_act_root("/tmp/kernel_act_root_v1")
    os.environ["BASS_ACT_ROOT_JSON_PATH"] = _root
else:
    _MERGE_TABLES = False

import concourse.bass as bass
import concourse.tile as tile
from concourse import bacc, mybir
from concourse.bass_utils import run_bass_kernel_spmd
from concourse.dve_ops import AFFINE_MUL_REDUCE, LN_BWD_DX_ANT

F32 = mybir.dt.float32
AF = mybir.ActivationFunctionType
ALU = mybir.AluOpType

# The Tile scheduler's cost model assumes the PE ramps to 2.4 GHz after
# 3us of continuous work; on this kernel the HAM never grants the warm
# clock (measured: all matmuls at ~1.2 GHz).  Pinning the modeled warm
# clock to 1.2 GHz realigns the scheduler's readiness estimates with
# reality (pure scheduling hint; no semantic change).
if os.environ.get("KERNEL_SCHED_COLD_PE", "1") == "1":
    from concourse.hw_specs import TRN2Spec as _Spec

    _Spec.PE_CYCLE = 1e9 / 1.2e9

if _MERGE_TABLES:
    # keep the bass-side view of the table sets consistent with the
    # custom act root (the returned dict is cached; mutate in place).
    from concourse.hw_specs import get_activation_tables

    _tabs = get_activation_tables("gen3")
    _sig = _tabs["sigmoid_and_others"]
    _sig.add(AF.Ln)
    _sig.discard(AF.Tanh)
    for _af in ("Erf", "Arctan"):
        if hasattr(AF, _af):
            _sig.discard(getattr(AF, _af))

# matmul operand dtype.  float32r (e8m11-rounded fp32) streams 1 col/cycle
# vs 4 cycles/col for full fp32; bfloat16 streams 2 cols/cycle.
_MM_MODE = os.environ.get("KERNEL_MM_DT", "bf16")
RD = {"f32r": mybir.dt.float32r, "bf16": mybir.dt.bfloat16, "f32": F32}[_MM_MODE]
RD16 = mybir.dt.bfloat16 if _MM_MODE == "bf16" else F32

_CACHE = {}


def _round_fp32r(a):
    a = np.ascontiguousarray(a, np.float32)
    if _MM_MODE == "f32":
        return a
    if _MM_MODE == "bf16":
        import ml_dtypes
        return np.ascontiguousarray(a.astype(ml_dtypes.bfloat16))
    bits = a.view(np.uint32)
    out = ((bits.astype(np.uint64) + 0x800) & 0xFFFFF000).astype(np.uint32)
    return out.view(np.float32)


def _build_program(with_b1h):
    nc = bacc.Bacc("TRN2", target_bir_lowering=False, debug=False)

    x_seq = nc.dram_tensor("x_seq", [T_ENC, BC], RD, kind="ExternalInput").ap()
    eps_seq = nc.dram_tensor("eps_seq", [GAMMA - 1, BC], F32, kind="ExternalInput").ap()
    r_w = nc.dram_tensor("r_w", [U, 3 * U], RD, kind="ExternalInput").ap()
    k_w = nc.dram_tensor("k_w", [RN, 3 * U], RD, kind="ExternalInput").ap()
    k_col = nc.dram_tensor("k_col", [U, 3], F32, kind="ExternalInput").ap()
    wkd0 = nc.dram_tensor("wkd0", [U, 2 * RN * GAMMA], RD, kind="ExternalInput").ap()
    wkd1 = nc.dram_tensor("wkd1", [U, 2 * RN * GAMMA], RD, kind="ExternalInput").ap()
    wb0 = nc.dram_tensor("wb0", [RN, GAMMA], F32, kind="ExternalInput").ap()
    cb1 = nc.dram_tensor("cb1", [RN, GAMMA], F32, kind="ExternalInput").ap()
    gb = nc.dram_tensor("gb", [U, 4], F32, kind="ExternalInput").ap()
    h0_z = nc.dram_tensor("h0_z", [U, BC], RD, kind="ExternalInput").ap()
    loc_fm = nc.dram_tensor("loc_fm", [GAMMA, BC], F32, kind="ExternalOutput").ap()
    sp_fm = nc.dram_tensor("sp_fm", [GAMMA, BC], F32, kind="ExternalOutput").ap()

    with tile.TileContext(nc) as tc, ExitStack() as es:
        consts = es.enter_context(tc.tile_pool(name="consts", bufs=1))
        R = consts.tile([U, 3 * U], RD)
        K = consts.tile([RN, 3 * U], RD)
        KC = consts.tile([U, 3], F32)
        WKD0 = consts.tile([U, 2 * RN * GAMMA], RD)
        WKD1 = consts.tile([U, 2 * RN * GAMMA], RD)
        WB0 = consts.tile([RN, GAMMA], F32)
        CB1 = consts.tile([RN, GAMMA], F32)
        GB = consts.tile([U, 4], F32)
        EPT = consts.tile([RN, CW], F32)
        nc.vector.memset(EPT[:], 0.0)
        nc.sync.dma_start(R[:], r_w[:])
        nc.sync.dma_start(K[:], k_w[:])
        nc.sync.dma_start(KC[:], k_col[:])
        nc.sync.dma_start(WKD0[:], wkd0[:])
        nc.sync.dma_start(WKD1[:], wkd1[:])
        nc.sync.dma_start(WB0[:], wb0[:])
        nc.sync.dma_start(CB1[:], cb1[:])
        nc.sync.dma_start(GB[:], gb[:])

        hpool = es.enter_context(tc.tile_pool(name="h", bufs=4))
        gates = es.enter_context(tc.tile_pool(name="gates", bufs=3))
        samp = es.enter_context(tc.tile_pool(name="samp", bufs=2))
        stage = es.enter_context(tc.tile_pool(name="stage", bufs=5))
        ps_g = es.enter_context(tc.tile_pool(
            name="psg", bufs=int(os.environ.get("KERNEL_PS_BUFS", "7")),
            space="PSUM"))
        ps_d = es.enter_context(tc.tile_pool(name="psd", bufs=1, space="PSUM"))
        DUM = ps_d.tile([U, 128], F32, tag="dum")
        n_dum_e = int(os.environ.get("KERNEL_DUMMY_ENC", "8"))
        n_dum_d = int(os.environ.get("KERNEL_DUMMY_DEC", "14"))

        def pe_warm(n):
            """Dependency-free 128-col junk matmuls into a scratch PSUM
            bank.  The HAM clock-gate only grants the 2.4 GHz PE clock
            while its activity window stays busy; these fill PE-idle gaps.
            128-col quanta (~107ns cold / ~55ns warm) keep the in-order
            queue block far smaller than the h2/y wait they fill (the
            v9 failure was 384-col x10 = 3.2us of blocking)."""
            for _ in range(n):
                nc.tensor.matmul(DUM[:], R[:, z3], R[:, 0:128],
                                 start=True, stop=True)

        hT = hpool.tile([U, BC], RD, tag="h")
        nc.sync.dma_start(hT[:], h0_z[:])
        hbox = [hT, None]  # [current h, next h being written]

        z3 = bass.ts(0, U)
        r3 = bass.ts(1, U)
        h3 = bass.ts(2, U)

        def gru_elem(c, psr, psh, psz, psx, xb):
            """Post-matmul element chain for chunk c ([U,CW] PSUM tiles).
            Either xb (encoder broadcast tile; x*K_h enters via STT) or
            psx (decoder; K_h@y PSUM)."""
            cs = bass.ts(c, CW)
            hc = hbox[0][:, cs]
            # r = sigmoid(rho + br)                     (GB1 = br)
            r_ = gates.tile([U, CW], RD16, tag=f"r_{c}")
            nc.scalar.activation(r_[:], psr[:], AF.Sigmoid,
                                 bias=GB[:, 1:2], scale=1.0)
            hrec = psh[:]
            if with_b1h:
                hb = gates.tile([U, CW], F32, tag=f"hb_{c}")
                nc.vector.tensor_scalar(
                    hb[:], psh[:], GB[:, 3:4], None, op0=ALU.add
                )
                hrec = hb[:]
            tt = gates.tile([U, CW], F32, tag=f"t_{c}")
            nc.vector.tensor_mul(tt[:], r_[:], hrec)
            uu = gates.tile([U, CW], F32, tag=f"u_{c}")
            if xb is not None:
                nc.vector.scalar_tensor_tensor(
                    uu[:], xb[:, cs], KC[:, 2:3], tt[:],
                    op0=ALU.mult, op1=ALU.add,
                )
            else:
                nc.vector.tensor_add(uu[:], tt[:], psx[:])
            hh = gates.tile([U, CW], RD16, tag=f"hh_{c}")
            nc.scalar.activation(hh[:], uu[:], AF.Tanh, bias=GB[:, 2:3],
                                 scale=1.0)
            # u1 = 1-z = sigmoid(-(zeta + bz))  (GB0 = -bz).  Emitted after
            # the tanh: u1 is needed only by e, and an earlier emission lets
            # the scheduler queue-block the tanh behind the other chunk's
            # sigmoids on the in-order ACT engine.
            u1 = gates.tile([U, CW], RD16, tag=f"u1_{c}")
            nc.scalar.activation(u1[:], psz[:], AF.Sigmoid,
                                 bias=GB[:, 0:1], scale=-1.0)
            # h' = h + (1-z)*(hh - h): three consecutive DVE ops (bf16
            # operands hit the 2x_1P packed mode)
            d = gates.tile([U, CW], RD16, tag=f"d_{c}")
            nc.vector.tensor_sub(d[:], hh[:], hc)
            e = gates.tile([U, CW], RD16, tag=f"e_{c}")
            nc.vector.tensor_mul(e[:], u1[:], d[:])
            nc.vector.tensor_add(hbox[1][:, cs], hc, e[:])

        # ---- encoder: 48 GRU steps ----
        for t in range(T_ENC):
            xb = stage.tile([U, BC], RD, tag="xb")
            nc.sync.dma_start(xb[:], x_seq[t : t + 1, :].partition_broadcast(U))
            hT = hbox[0]
            h_next = hpool.tile([U, BC], RD, tag="h")
            hbox[1] = h_next
            for c in range(NCH):
                cs = bass.ts(c, CW)
                x_row = xb[0:1, cs]
                psr = ps_g.tile([U, CW], F32, tag="ps")
                psh = ps_g.tile([U, CW], F32, tag="ps")
                psz = ps_g.tile([U, CW], F32, tag="ps")
                nc.tensor.matmul(psr[:], K[0:1, r3], x_row, start=True, stop=False)
                nc.tensor.matmul(psr[:], R[:, r3], hT[:, cs], start=False, stop=True)
                nc.tensor.matmul(psh[:], R[:, h3], hT[:, cs], start=True, stop=True)
                nc.tensor.matmul(psz[:], K[0:1, z3], x_row, start=True, stop=False)
                nc.tensor.matmul(psz[:], R[:, z3], hT[:, cs], start=False, stop=True)
                gru_elem(c, psr, psh, psz, None, xb)
            pe_warm(n_dum_e)
            hbox[0] = hbox[1]

        # ---- decoder ----
        def dense_mms(t):
            """Dense head matmuls for step t.  Both chunks land in one
            [2, CW] PSUM tile (row c = chunk c) via zero-padded lhsT
            columns accumulated across two matmuls, so every downstream
            row op handles both chunks in a single instruction."""
            hT = hbox[0]
            psl = ps_g.tile([RN, CW], F32, tag="ps")
            pss = ps_g.tile([RN, CW], F32, tag="ps")
            for c, WKD in ((0, WKD0), (1, WKD1)):
                cs = bass.ts(c, CW)
                o = 2 * RN * t
                nc.tensor.matmul(psl[:], WKD[:, o : o + RN], hT[:, cs],
                                 start=(c == 0), stop=(c == 1))
                nc.tensor.matmul(pss[:], WKD[:, o + RN : o + 2 * RN],
                                 hT[:, cs], start=(c == 0), stop=(c == 1))
            return psl, pss

        def head_rows(t, ps):
            """sigmoid+ln+loc rows for step t ([2, CW]: row c = chunk c);
            DMA out; returns (loc, sp)."""
            psl, pss = ps
            g = samp.tile([RN, CW], F32, tag="g")
            loc = samp.tile([RN, CW], F32, tag="loc")
            # g = sigmoid(-(s + C + wb1))   (CB1 = -(C+wb1))
            nc.scalar.activation(
                g[:], pss[:], AF.Sigmoid, bias=CB1[:, t : t + 1],
                scale=-1.0,
            )
            # loc = h@W0 + wb0   (DVE: keeps the ACT queue free for the
            # table-load + Ln chain)
            nc.vector.tensor_scalar(
                loc[:], psl[:], WB0[:, t : t + 1], None, op0=ALU.add,
            )
            # sp = ln(g) = -softplus  (host applies 1e-5 - 0.05*sp)
            sp = samp.tile([RN, CW], F32, tag="sp")
            nc.scalar.activation(sp[:], g[:], AF.Ln, bias=0.0, scale=1.0)
            for c in range(NCH):
                cs = bass.ts(c, CW)
                rr = c * RP
                nc.sync.dma_start(loc_fm[t : t + 1, cs], loc[rr : rr + 1, :])
                nc.sync.dma_start(sp_fm[t : t + 1, cs], sp[rr : rr + 1, :])
            return loc, sp

        ps = dense_mms(0)
        for t in range(1, GAMMA):
            # next step's recurrent matmuls first: PE works through the tail
            hT = hbox[0]
            gps = []
            for c in range(NCH):
                cs = bass.ts(c, CW)
                psr = ps_g.tile([U, CW], F32, tag="ps")
                psh = ps_g.tile([U, CW], F32, tag="ps")
                psz = ps_g.tile([U, CW], F32, tag="ps")
                psx = ps_g.tile([U, CW], F32, tag="ps")
                nc.tensor.matmul(psr[:], R[:, r3], hT[:, cs], start=True,
                                 stop=False)
                nc.tensor.matmul(psh[:], R[:, h3], hT[:, cs], start=True,
                                 stop=True)
                nc.tensor.matmul(psz[:], R[:, z3], hT[:, cs], start=True,
                                 stop=False)
                gps.append((psr, psh, psz, psx))
            # before K@y in the queue, which waits on y anyway: these can
            # never delay real work, only bridge the sampling-tail idle
            pe_warm(n_dum_d)
            loc, sp = head_rows(t - 1, ps)
            # sample: y = loc + (1e-5 - 0.05*sp) * eps, rows {0, RP}
            ep = EPT
            for c in range(NCH):
                cs = bass.ts(c, CW)
                rr = c * RP
                nc.sync.dma_start(ep[rr : rr + 1, :], eps_seq[t - 1 : t, cs])
            m = samp.tile([RN, CW], F32, tag="m")
            nc.vector._custom_dve(
                AFFINE_MUL_REDUCE, out=m[:], in0=sp[:], in1=ep[:],
                s0=-OP_SCALE, s1=1e-5,
            )
            y = samp.tile([RN, CW], RD, tag="y")
            nc.vector.tensor_add(y[:], m[:], loc[:])
            h_next = hpool.tile([U, BC], RD, tag="h")
            hbox[1] = h_next
            for c in range(NCH):
                psr, psh, psz, psx = gps[c]
                rr = c * RP
                y_row = y[rr : rr + 1, :]
                Kc = K[rr : rr + 1, :]
                nc.tensor.matmul(psr[:], Kc[:, r3], y_row, start=False,
                                 stop=True)
                nc.tensor.matmul(psx[:], Kc[:, h3], y_row, start=True,
                                 stop=True)
                nc.tensor.matmul(psz[:], Kc[:, z3], y_row, start=False,
                                 stop=True)
                gru_elem(c, psr, psh, psz, psx, None)
            hbox[0] = hbox[1]
            ps = dense_mms(t)
        head_rows(GAMMA - 1, ps)

    nc.compile()
    return nc


def _k_rows(gru_kernel):
    k = np.zeros((RN, 3 * U), np.float32)
    k[0] = np.asarray(gru_kernel, np.float32)[0]
    k[RP] = k[0]
    return k


def _host_prep(inputs, gru_kernel, gru_rec_kernel, gru_bias, dv_loc, dv_rho,
               dv_eps, samp_eps):
    inputs = np.asarray(inputs, np.float32)
    B = inputs.shape[0]
    assert B == B_FULL, f"kernel compiled for B={B_FULL}, got {B}"
    xT = _round_fp32r(inputs[:, :T_ENC, 0].T)                  # [48, B]
    epsT = np.ascontiguousarray(np.asarray(samp_eps, np.float32)[:, :, 0])

    gru_bias = np.asarray(gru_bias, np.float32)
    b0, b1 = gru_bias[0], gru_bias[1]
    gb = np.zeros((U, 4), np.float32)
    gb[:, 0] = -(b0[0:U] + b1[0:U])
    gb[:, 1] = b0[U : 2 * U] + b1[U : 2 * U]
    gb[:, 2] = b0[2 * U : 3 * U]
    gb[:, 3] = b1[2 * U : 3 * U]

    dv_loc = np.asarray(dv_loc, np.float32)
    dv_rho = np.asarray(dv_rho, np.float32)
    dv_eps = np.asarray(dv_eps, np.float32)
    scale_q = np.float32(1e-5) + np.float32(Q_SCALE) * np.logaddexp(
        np.float32(C_SP) + dv_rho, np.float32(0.0), dtype=np.float32
    )
    w_all = dv_loc[None, :] + scale_q[None, :] * dv_eps        # [28, 258]
    wk = w_all[:, : 2 * U].reshape(GAMMA, U, 2).transpose(1, 0, 2)  # [U,28,2]
    # zero-padded dense lhsT pairs: chunk c writes row c of the [2, CW]
    # loc/scale PSUM tiles.  wkd_c[:, 4t:4t+2] = loc cols, 4t+2:4t+4 = scale.
    wkd0 = np.zeros((U, 2 * RN * GAMMA), np.float32)
    wkd1 = np.zeros((U, 2 * RN * GAMMA), np.float32)
    wkd0[:, 0 :: 2 * RN] = wk[:, :, 0]          # loc, chunk0 -> row 0
    wkd1[:, RP :: 2 * RN] = wk[:, :, 0]         # loc, chunk1 -> row RP
    wkd0[:, RN :: 2 * RN] = wk[:, :, 1]         # scale, chunk0
    wkd1[:, RN + RP :: 2 * RN] = wk[:, :, 1]    # scale, chunk1
    wb0 = np.broadcast_to(w_all[:, 2 * U][None, :], (RN, GAMMA))
    cb1 = np.broadcast_to(
        (-(np.float32(C_SP) + w_all[:, 2 * U + 1]))[None, :], (RN, GAMMA)
    )  # negated: softplus comes via -ln(sigmoid(-x))

    shared = {
        "r_w": _round_fp32r(gru_rec_kernel),
        "k_w": _round_fp32r(_k_rows(gru_kernel)),
        "k_col": np.ascontiguousarray(
            np.asarray(gru_kernel, np.float32).reshape(3, U).T
        ),
        "wkd0": _round_fp32r(wkd0),
        "wkd1": _round_fp32r(wkd1),
        "wb0": np.ascontiguousarray(wb0, np.float32),
        "cb1": np.ascontiguousarray(cb1, np.float32),
        "gb": gb,
        "h0_z": _round_fp32r(np.zeros((U, BC), np.float32)),
    }
    in_maps = []
    for c in range(N_CORES):
        sl = slice(c * BC, (c + 1) * BC)
        in_maps.append(
            dict(
                shared,
                x_seq=np.ascontiguousarray(xT[:, sl]),
                eps_seq=np.ascontiguousarray(epsT[:, sl]),
            )
        )
    return in_maps, bool(np.any(gb[:, 3] != 0.0))


def _get_nc(with_b1h=False):
    key = ("nc", with_b1h)
    if key not in _CACHE:
        _CACHE[key] = _build_program(with_b1h)
    return _CACHE[key]


def run(inputs_dict, trace=False, trace_kwargs=None):
    in_maps, with_b1h = _host_prep(**inputs_dict)
    nc = _get_nc(with_b1h)
    res = run_bass_kernel_spmd(
        nc, in_maps, list(range(N_CORES)), trace=trace,
        **(trace_kwargs or {}),
    )
    _CACHE["last_results"] = res
    out = np.empty((B_FULL, GAMMA, 2), np.float32)
    for c in range(N_CORES):
        loc = res.results[c]["loc_fm"]                         # [28, 1024]
        sp = res.results[c]["sp_fm"]                           # [28, 1024]
        sc = np.float32(1e-5) - np.float32(OP_SCALE) * sp
        out[c * BC : (c + 1) * BC, :, 0] = loc.T
        out[c * BC : (c + 1) * BC, :, 1] = sc.T
    return out


def kernel(**inputs):
    return run(inputs, trace=bool(os.environ.get("KERNEL_TRACE")))
